# revision 2
# baseline (speedup 1.0000x reference)
"""Trainium2 Bass kernel: GroupNorm(32) + single-head self-attention block + residual.

Computation (per image):
    h  = group_norm(x)                          x: [C=512, HW=1024] channel-major
    qT = wq @ h + bq ; kT = wk @ h + bk         [C, HW] channel-major
    v  = (h.T @ wv.T)                           [HW, C] token-major
    sT = kT.T-contraction -> S^T[m, n]          [HW, HW] (scores transposed)
    p  = exp(sT / sqrt(C)) ; denom[n] = sum_m p
    aT = (v.T-contraction over m) * (1/denom)   [C, HW] channel-major
    y  = wo @ aT + (bo + wo@bv) + x             [C, HW]

Sharding: data-parallel over batch; 8 cores x 4 images. Weights replicated.
All heavy matmuls run as float32r (full-speed PE, ~1e-4 relative rounding).
"""

import math
import os

import numpy as np

import concourse.bass as bass
import concourse.tile as tile
from concourse import bacc, mybir
from concourse.bass_utils import run_bass_kernel_spmd

N_CORES = 8
B, C, H, W = 32, 512, 32, 32
HW = H * W                      # 1024 tokens
BL = B // N_CORES               # 4 images per core
NGRP = 32                       # groupnorm groups
GS = C // NGRP                  # 16 channels per group
EPS = 1e-5
P = 128
NT = C // P                     # 4 channel partition-tiles
MT = HW // P                    # 8 token partition-tiles
FCH = 512                       # matmul moving free-dim chunk (one PSUM bank fp32)
NCH = HW // FCH                 # 2 free chunks per 1024
F32 = mybir.dt.float32
F32R = mybir.dt.float32r
SCALE = 1.0 / math.sqrt(C)

ACT_EXP = mybir.ActivationFunctionType.Exp
ACT_SQRT = mybir.ActivationFunctionType.Sqrt
OP_ADD = mybir.AluOpType.add
OP_MULT = mybir.AluOpType.mult
OP_SUB = mybir.AluOpType.subtract

LAST_EXEC_NS = None
_CACHED_NC = None


def _build_nc():
    from contextlib import ExitStack

    nc = bacc.Bacc("TRN2", target_bir_lowering=False, debug=False)

    x_d = nc.dram_tensor("x", [BL, C, HW], F32, kind="ExternalInput").ap()
    wqT_d = nc.dram_tensor("wqT", [C, C], F32, kind="ExternalInput").ap()
    wkT_d = nc.dram_tensor("wkT", [C, C], F32, kind="ExternalInput").ap()
    wvT_d = nc.dram_tensor("wvT", [C, C], F32, kind="ExternalInput").ap()
    woT_d = nc.dram_tensor("woT", [C, C], F32, kind="ExternalInput").ap()
    bq_d = nc.dram_tensor("bq", [C], F32, kind="ExternalInput").ap()
    bk_d = nc.dram_tensor("bk", [C], F32, kind="ExternalInput").ap()
    boP_d = nc.dram_tensor("boP", [C], F32, kind="ExternalInput").ap()
    gw_d = nc.dram_tensor("gw", [C], F32, kind="ExternalInput").ap()
    gb_d = nc.dram_tensor("gb", [C], F32, kind="ExternalInput").ap()
    gm_d = nc.dram_tensor("gm", [P, NT, NGRP], F32, kind="ExternalInput").ap()
    gmt_d = nc.dram_tensor("gmt", [NGRP, NT, P], F32, kind="ExternalInput").ap()
    ones_d = nc.dram_tensor("ones", [P, P], F32, kind="ExternalInput").ap()
    y_d = nc.dram_tensor("y", [BL, C, HW], F32, kind="ExternalOutput").ap()

    x_r = x_d.rearrange("b (t p) n -> b t p n", p=P)
    y_r = y_d.rearrange("b (t p) n -> b t p n", p=P)

    with tile.TileContext(nc) as tc, ExitStack() as ctx:
        pool = lambda name, bufs, space="SBUF": ctx.enter_context(
            tc.tile_pool(name=name, bufs=bufs, space=space)
        )
        p_const = pool("const", 1)
        p_stage = pool("stage", 2)
        p_x = pool("x", 5)
        p_X = pool("X", 5)
        p_qt = pool("qt", NT)
        p_kt = pool("kt", NT)
        p_v = pool("v", MT)
        p_exp = pool("exp", MT)
        p_a = pool("a", NT)
        p_recip = pool("recip", 2)
        p_out = pool("out", 3)
        p_small = pool("small", 4)
        ps_proj = pool("ps_proj", 4, space="PSUM")
        ps_big = pool("ps_big", 2, space="PSUM")

        # ---- constants / weights (one-time) ----
        def load_wT(dram):
            t_r = p_const.tile([P, NT, C], F32R, tag=f"w_{dram.name}")
            r = dram.rearrange("(t p) o -> t p o", p=P)
            for ci in range(NT):
                st = p_stage.tile([P, C], F32, tag="wstage")
                nc.sync.dma_start(out=st[:], in_=r[ci])
                nc.vector.tensor_copy(out=t_r[:, ci, :], in_=st[:])
            return t_r

        wq_r = load_wT(wqT_d)
        wk_r = load_wT(wkT_d)
        wv_r = load_wT(wvT_d)
        wo_r = load_wT(woT_d)

        def load_cols(dram, tag):
            t = p_const.tile([P, NT], F32, tag=tag)
            nc.sync.dma_start(out=t[:], in_=dram.rearrange("(t p) -> p t", p=P))
            return t

        bq_sb = load_cols(bq_d, "bq")
        bk_sb = load_cols(bk_d, "bk")
        boP_sb = load_cols(boP_d, "boP")
        gw_sb = load_cols(gw_d, "gw")
        gb_sb = load_cols(gb_d, "gb")

        gm_sb = p_const.tile([P, NT, NGRP], F32, tag="gm")
        nc.sync.dma_start(out=gm_sb[:], in_=gm_d)
        gmt_sb = p_const.tile([NGRP, NT, P], F32, tag="gmt")
        nc.sync.dma_start(out=gmt_sb[:], in_=gmt_d)

        ones_f = p_const.tile([P, P], F32, tag="ones_f")
        nc.sync.dma_start(out=ones_f[:], in_=ones_d)
        ones_r = p_const.tile([P, P], F32R, tag="ones_r")
        nc.vector.tensor_copy(out=ones_r[:], in_=ones_f[:])

        eps_sb = p_const.tile([P, 1], F32, tag="eps")
        nc.vector.memset(eps_sb[:], EPS)

        # ---- per-image pipeline ----
        for b in range(BL):
            # load x tiles (channel-major [c, n])
            xt = []
            for t in range(NT):
                xtile = p_x.tile([P, HW], F32, tag="x")
                nc.sync.dma_start(out=xtile[:], in_=x_r[b, t])
                xt.append(xtile)

            # ---- groupnorm stats: bn_stats per c-tile, group-reduce via matmul
            psg = ps_proj.tile([NGRP, 2], F32, tag="proj")
            for t in range(NT):
                st = p_small.tile([P, NCH, 6], F32, tag="bnst")
                for i in range(NCH):
                    nc.vector.bn_stats(
                        out=st[:, i, :], in_=xt[t][:, i * FCH : (i + 1) * FCH]
                    )
                mv = p_small.tile([P, 2], F32, tag="bnmv")
                nc.vector.bn_aggr(out=mv[:], in_=st[:])
                # stat2 = [mean, var + mean^2]
                stat2 = p_small.tile([P, 2], F32, tag="stat2")
                nc.vector.tensor_copy(out=stat2[:, 0:1], in_=mv[:, 0:1])
                m2 = p_small.tile([P, 1], F32, tag="m2")
                nc.vector.tensor_mul(m2[:], mv[:, 0:1], mv[:, 0:1])
                nc.vector.tensor_add(stat2[:, 1:2], mv[:, 1:2], m2[:])
                nc.tensor.matmul(
                    psg[:], gm_sb[:, t, :], stat2[:], start=(t == 0), stop=(t == NT - 1)
                )

            # group mean / rstd  (gmr: [32, (mean, rstd)])
            gmr = p_small.tile([NGRP, 2], F32, tag="gmr")
            nc.vector.tensor_scalar_mul(gmr[:, 0:1], psg[:, 0:1], 1.0 / GS)
            e2g = p_small.tile([NGRP, 1], F32, tag="e2g")
            nc.vector.tensor_scalar_mul(e2g[:], psg[:, 1:2], 1.0 / GS)
            m2g = p_small.tile([NGRP, 1], F32, tag="m2g")
            nc.vector.tensor_mul(m2g[:], gmr[:, 0:1], gmr[:, 0:1])
            varg = p_small.tile([NGRP, 1], F32, tag="varg")
            nc.vector.tensor_sub(varg[:], e2g[:], m2g[:])
            sdg = p_small.tile([NGRP, 1], F32, tag="sdg")
            nc.scalar.activation(
                out=sdg[:], in_=varg[:], func=ACT_SQRT, bias=eps_sb[0:NGRP, :]
            )
            nc.vector.reciprocal(out=gmr[:, 1:2], in_=sdg[:])

            # normalize + affine -> X (fp32r, channel-major)
            Xr = []
            for t in range(NT):
                psb = ps_proj.tile([P, 2], F32, tag="proj")
                nc.tensor.matmul(psb[:], gmt_sb[:, t, :], gmr[:], start=True, stop=True)
                acol = p_small.tile([P, 1], F32, tag="acol")
                nc.vector.tensor_mul(acol[:], psb[:, 1:2], gw_sb[:, t : t + 1])
                tmb = p_small.tile([P, 1], F32, tag="tmb")
                nc.vector.tensor_mul(tmb[:], psb[:, 0:1], acol[:])
                bcol = p_small.tile([P, 1], F32, tag="bcol")
                nc.vector.tensor_sub(bcol[:], gb_sb[:, t : t + 1], tmb[:])
                Xt = p_X.tile([P, HW], F32R, tag="X")
                nc.vector.tensor_scalar(
                    out=Xt[:], in0=xt[t][:], scalar1=acol[:], scalar2=bcol[:],
                    op0=OP_MULT, op1=OP_ADD,
                )
                Xr.append(Xt)

            # ---- Q^T / K^T projections (channel-major [o, n])
            def proj_cm(w_r, bias_sb, tag, out_pool):
                outs = []
                for ot in range(NT):
                    dst = out_pool.tile([P, HW], F32R, tag=tag)
                    for nch in range(NCH):
                        ps = ps_proj.tile([P, FCH], F32, tag="proj")
                        for ci in range(NT):
                            nc.tensor.matmul(
                                ps[:],
                                w_r[:, ci, ot * P : (ot + 1) * P],
                                Xr[ci][:, nch * FCH : (nch + 1) * FCH],
                                start=(ci == 0),
                                stop=(ci == NT - 1),
                            )
                        nc.vector.tensor_scalar(
                            out=dst[:, nch * FCH : (nch + 1) * FCH], in0=ps[:],
                            scalar1=bias_sb[:, ot : ot + 1], scalar2=None, op0=OP_ADD,
                        )
                    outs.append(dst)
                return outs

            QT = proj_cm(wq_r, bq_sb, "qt", p_qt)
            KT = proj_cm(wk_r, bk_sb, "kt", p_kt)

            # ---- V projection (token-major [m, o]); bias bv folded into boP
            Vr = []
            for mt in range(MT):
                ps = ps_proj.tile([P, FCH], F32, tag="proj")
                for ci in range(NT):
                    nc.tensor.matmul(
                        ps[:],
                        Xr[ci][:, mt * P : (mt + 1) * P],
                        wv_r[:, ci, :],
                        start=(ci == 0),
                        stop=(ci == NT - 1),
                    )
                vt = p_v.tile([P, C], F32R, tag="v")
                nc.vector.tensor_copy(out=vt[:], in_=ps[:])
                Vr.append(vt)

            # ---- scores S^T[m, n] -> exp -> column sums (denominator)
            psC = []
            for _nch in range(NCH):
                psc_t = ps_proj.tile([P, FCH], F32, tag="proj", name=f"psc_{b}_{_nch}")
                psC.append(psc_t)
            expT = []
            for mt in range(MT):
                psS = ps_big.tile([P, HW], F32, tag="big")
                for nch in range(NCH):
                    for ci in range(NT):
                        nc.tensor.matmul(
                            psS[:, nch * FCH : (nch + 1) * FCH],
                            KT[ci][:, mt * P : (mt + 1) * P],
                            QT[ci][:, nch * FCH : (nch + 1) * FCH],
                            start=(ci == 0),
                            stop=(ci == NT - 1),
                        )
                et = p_exp.tile([P, HW], F32R, tag="exp")
                nc.scalar.activation(out=et[:], in_=psS[:], func=ACT_EXP, scale=SCALE)
                for nch in range(NCH):
                    nc.tensor.matmul(
                        psC[nch][:],
                        ones_r[:],
                        et[:, nch * FCH : (nch + 1) * FCH],
                        start=(mt == 0),
                        stop=(mt == MT - 1),
                    )
                expT.append(et)

            recip = p_recip.tile([P, HW], F32, tag="recip")
            for nch in range(NCH):
                nc.vector.reciprocal(
                    out=recip[:, nch * FCH : (nch + 1) * FCH], in_=psC[nch][:]
                )

            # ---- A^T[c, n] = V^T-contraction over m, normalized by 1/denom
            Ar = []
            for c2 in range(NT):
                psA = ps_big.tile([P, HW], F32, tag="big")
                for nch in range(NCH):
                    for mt in range(MT):
                        nc.tensor.matmul(
                            psA[:, nch * FCH : (nch + 1) * FCH],
                            Vr[mt][:, c2 * P : (c2 + 1) * P],
                            expT[mt][:, nch * FCH : (nch + 1) * FCH],
                            start=(mt == 0),
                            stop=(mt == MT - 1),
                        )
                at = p_a.tile([P, HW], F32R, tag="a")
                nc.vector.tensor_mul(at[:], psA[:], recip[:])
                Ar.append(at)

            # ---- output projection + bias + residual
            for co in range(NT):
                for nch in range(NCH):
                    ps = ps_proj.tile([P, FCH], F32, tag="proj")
                    for oi in range(NT):
                        nc.tensor.matmul(
                            ps[:],
                            wo_r[:, oi, co * P : (co + 1) * P],
                            Ar[oi][:, nch * FCH : (nch + 1) * FCH],
                            start=(oi == 0),
                            stop=(oi == NT - 1),
                        )
                    ot = p_out.tile([P, FCH], F32, tag="out")
                    nc.vector.scalar_tensor_tensor(
                        out=ot[:], in0=ps[:], scalar=boP_sb[:, co : co + 1],
                        in1=xt[co][:, nch * FCH : (nch + 1) * FCH],
                        op0=OP_ADD, op1=OP_ADD,
                    )
                    nc.sync.dma_start(
                        out=y_r[b, co][:, nch * FCH : (nch + 1) * FCH], in_=ot[:]
                    )

    nc.compile()
    return nc


def _host_inputs(x, gn_scale, gn_bias, wq, bq, wk, bk, wv, bv, wo, bo):
    f = lambda a: np.ascontiguousarray(np.asarray(a, dtype=np.float32))
    x = f(x).reshape(B, C, HW)
    boP = f(bo) + f(wo) @ f(bv)

    gm = np.zeros((P, NT, NGRP), np.float32)
    gmt = np.zeros((NGRP, NT, P), np.float32)
    for t in range(NT):
        for p in range(P):
            g = (t * P + p) // GS
            gm[p, t, g] = 1.0
            gmt[g, t, p] = 1.0
    ones = np.ones((P, P), np.float32)

    shared = {
        "wqT": np.ascontiguousarray(f(wq).T),
        "wkT": np.ascontiguousarray(f(wk).T),
        "wvT": np.ascontiguousarray(f(wv).T),
        "woT": np.ascontiguousarray(f(wo).T),
        "bq": f(bq), "bk": f(bk), "boP": boP,
        "gw": f(gn_scale), "gb": f(gn_bias),
        "gm": gm, "gmt": gmt, "ones": ones,
    }
    in_maps = []
    for i in range(N_CORES):
        m = dict(shared)
        m["x"] = np.ascontiguousarray(x[i * BL : (i + 1) * BL])
        in_maps.append(m)
    return in_maps


def kernel(x, gn_scale, gn_bias, wq, bq, wk, bk, wv, bv, wo, bo):
    global _CACHED_NC, LAST_EXEC_NS
    assert x.shape == (B, C, H, W)
    if _CACHED_NC is None:
        _CACHED_NC = _build_nc()
    in_maps = _host_inputs(x, gn_scale, gn_bias, wq, bq, wk, bk, wv, bv, wo, bo)
    trace = os.environ.get("ATT_TRACE", "0") == "1"
    kwargs = {}
    tdir = os.environ.get("ATT_TRACE_DIR")
    if tdir:
        kwargs["tmpdir"] = tdir
    res = run_bass_kernel_spmd(
        _CACHED_NC, in_maps, core_ids=list(range(N_CORES)), trace=trace, **kwargs
    )
    LAST_EXEC_NS = res.exec_time_ns
    y = np.concatenate([res.results[i]["y"] for i in range(N_CORES)], axis=0)
    return y.reshape(B, C, H, W).astype(np.float32)


# revision 3
# speedup vs baseline: 1.0713x; 1.0713x over previous
"""Trainium2 Bass kernel: GroupNorm(32) + single-head self-attention block + residual.

Computation (per image, channel-major layouts):
    h  = group_norm(x)                         [C=512, HW=1024]
    qT = wq @ h + bq ; kT = wk @ h + bk        [C, HW]
    v  = h.T @ wv.T                            [HW, C] token-major
    sT[m, n] = sum_o kT[o,m] qT[o,n]           scores transposed
    p = exp(sT / sqrt(C)); denom[n] = sum_m p  (softmax w/o max-subtract: scores ~N(0,1))
    aT[c, n] = (sum_m v[m,c] p[m,n]) / denom[n]
    y  = wo @ aT + (bo + wo@bv) + x            [C, HW]

Sharding: data-parallel over batch; 8 cores x 4 images each. Weights replicated.
Heavy matmuls run as float32r (full-speed PE, ~1e-4 relative rounding error).
GroupNorm stats/broadcast use tiny fp32 matmuls with group-selector matrices.
The GN phase for image b+1 is emitted before image b's heavy phases so its
DVE/PE work schedules into image b's shadow (Tile keeps static per-engine order).
"""

import math
import os

import numpy as np

import concourse.bass as bass
import concourse.tile as tile
from concourse import bacc, mybir
from concourse.bass_utils import run_bass_kernel_spmd

N_CORES = 8
B, C, H, W = 32, 512, 32, 32
HW = H * W                      # 1024 tokens
BL = B // N_CORES               # 4 images per core
NGRP = 32                       # groupnorm groups
GS = C // NGRP                  # 16 channels per group
EPS = 1e-5
P = 128
NT = C // P                     # 4 channel partition-tiles
MT = HW // P                    # 8 token partition-tiles
FCH = 512                       # moving free-dim chunk (one PSUM bank fp32)
NCH = HW // FCH                 # 2 free chunks per 1024
F32 = mybir.dt.float32
F32R = mybir.dt.float32r
SCALE = 1.0 / math.sqrt(C)

ACT_EXP = mybir.ActivationFunctionType.Exp
ACT_SQRT = mybir.ActivationFunctionType.Sqrt
ACT_IDENT = mybir.ActivationFunctionType.Identity
OP_ADD = mybir.AluOpType.add
OP_MULT = mybir.AluOpType.mult

LAST_EXEC_NS = None
_CACHED_NC = None


def _build_nc():
    from contextlib import ExitStack

    nc = bacc.Bacc("TRN2", target_bir_lowering=False, debug=False)

    x_d = nc.dram_tensor("x", [BL, C, HW], F32, kind="ExternalInput").ap()
    wqT_d = nc.dram_tensor("wqT", [C, C], F32, kind="ExternalInput").ap()
    wkT_d = nc.dram_tensor("wkT", [C, C], F32, kind="ExternalInput").ap()
    wvT_d = nc.dram_tensor("wvT", [C, C], F32, kind="ExternalInput").ap()
    woT_d = nc.dram_tensor("woT", [C, C], F32, kind="ExternalInput").ap()
    bq_d = nc.dram_tensor("bq", [C], F32, kind="ExternalInput").ap()
    bk_d = nc.dram_tensor("bk", [C], F32, kind="ExternalInput").ap()
    boP_d = nc.dram_tensor("boP", [C], F32, kind="ExternalInput").ap()
    gw_d = nc.dram_tensor("gw", [C], F32, kind="ExternalInput").ap()
    gb_d = nc.dram_tensor("gb", [C], F32, kind="ExternalInput").ap()
    gm_d = nc.dram_tensor("gm", [P, NT, NGRP], F32, kind="ExternalInput").ap()
    gmt_d = nc.dram_tensor("gmt", [NGRP, NT, P], F32, kind="ExternalInput").ap()
    ones_d = nc.dram_tensor("ones", [P, P], F32, kind="ExternalInput").ap()
    y_d = nc.dram_tensor("y", [BL, C, HW], F32, kind="ExternalOutput").ap()

    x_r = x_d.rearrange("b (t p) n -> b t p n", p=P)
    y_r = y_d.rearrange("b (t p) n -> b t p n", p=P)

    ib = lambda k, d: int(os.environ.get(k, d))  # buf-count knobs for tuning
    with tile.TileContext(nc) as tc, ExitStack() as ctx:
        pool = lambda name, bufs, space="SBUF": ctx.enter_context(
            tc.tile_pool(name=name, bufs=bufs, space=space)
        )
        p_const = pool("const", 1)
        p_stage = pool("stage", ib("BUF_STAGE", 2))
        p_x = pool("x", ib("BUF_X", 8))
        p_X = pool("X", ib("BUF_XN", 7))
        p_qt = pool("qt", NT)
        p_kt = pool("kt", NT)
        p_v = pool("v", ib("BUF_V", 8))
        p_exp = pool("exp", ib("BUF_EXP", 8))
        p_a = pool("a", NT)
        p_recip = pool("recip", 2)
        p_out = pool("out", ib("BUF_OUT", 2))
        p_small = pool("small", 4)
        psum = pool("psum", ib("BUF_PSUM", 8), space="PSUM")

        def ps_tile(name, parts=P, free=FCH):
            return psum.tile([parts, free], F32, tag="u", name=name)

        # ---- small constants ----
        def load_cols(dram, tag):
            t = p_const.tile([P, NT], F32, tag=tag)
            nc.sync.dma_start(out=t[:], in_=dram.rearrange("(t p) -> p t", p=P))
            return t

        bq_sb = load_cols(bq_d, "bq")
        bk_sb = load_cols(bk_d, "bk")
        boP_sb = load_cols(boP_d, "boP")
        gw_sb = load_cols(gw_d, "gw")
        gb_sb = load_cols(gb_d, "gb")

        gm_sb = p_const.tile([P, NT, NGRP], F32, tag="gm")
        nc.sync.dma_start(out=gm_sb[:], in_=gm_d)
        gmt_sb = p_const.tile([NGRP, NT, P], F32, tag="gmt")
        nc.sync.dma_start(out=gmt_sb[:], in_=gmt_d)
        eps_sb = p_const.tile([P, 1], F32, tag="eps")
        nc.vector.memset(eps_sb[:], EPS)

        # ---- groupnorm phase (stats + normalize); emitted one image ahead ----
        def emit_gn(b):
            xt = []
            for t in range(NT):
                xtile = p_x.tile([P, HW], F32, tag="x", name=f"x_{b}_{t}")
                nc.sync.dma_start(out=xtile[:], in_=x_r[b, t])
                xt.append(xtile)

            psg = ps_tile(f"psg_{b}", parts=NGRP, free=2)
            for t in range(NT):
                st = p_small.tile([P, NCH, 6], F32, tag="bnst")
                for i in range(NCH):
                    nc.vector.bn_stats(
                        out=st[:, i, :], in_=xt[t][:, i * FCH : (i + 1) * FCH]
                    )
                mv = p_small.tile([P, 2], F32, tag="bnmv")
                nc.vector.bn_aggr(out=mv[:], in_=st[:])
                # stat2 = [mean, var + mean^2]
                stat2 = p_small.tile([P, 2], F32, tag="stat2")
                nc.vector.tensor_copy(out=stat2[:, 0:1], in_=mv[:, 0:1])
                m2 = p_small.tile([P, 1], F32, tag="m2")
                nc.vector.tensor_mul(m2[:], mv[:, 0:1], mv[:, 0:1])
                nc.vector.tensor_add(stat2[:, 1:2], mv[:, 1:2], m2[:])
                nc.tensor.matmul(
                    psg[:], gm_sb[:, t, :], stat2[:], start=(t == 0), stop=(t == NT - 1)
                )

            # gmr: [32 groups, (mean, rstd)]
            gmr = p_small.tile([NGRP, 2], F32, tag="gmr")
            nc.vector.tensor_scalar_mul(gmr[:, 0:1], psg[:, 0:1], 1.0 / GS)
            e2g = p_small.tile([NGRP, 1], F32, tag="e2g")
            nc.vector.tensor_scalar_mul(e2g[:], psg[:, 1:2], 1.0 / GS)
            m2g = p_small.tile([NGRP, 1], F32, tag="m2g")
            nc.vector.tensor_mul(m2g[:], gmr[:, 0:1], gmr[:, 0:1])
            varg = p_small.tile([NGRP, 1], F32, tag="varg")
            nc.vector.tensor_sub(varg[:], e2g[:], m2g[:])
            sdg = p_small.tile([NGRP, 1], F32, tag="sdg")
            nc.scalar.activation(
                out=sdg[:], in_=varg[:], func=ACT_SQRT, bias=eps_sb[0:NGRP, :]
            )
            nc.vector.reciprocal(out=gmr[:, 1:2], in_=sdg[:])

            Xr = []
            for t in range(NT):
                psb = ps_tile(f"psb_{b}_{t}", free=2)
                nc.tensor.matmul(psb[:], gmt_sb[:, t, :], gmr[:], start=True, stop=True)
                acol = p_small.tile([P, 1], F32, tag="acol")
                nc.vector.tensor_mul(acol[:], psb[:, 1:2], gw_sb[:, t : t + 1])
                tmb = p_small.tile([P, 1], F32, tag="tmb")
                nc.vector.tensor_mul(tmb[:], psb[:, 0:1], acol[:])
                bcol = p_small.tile([P, 1], F32, tag="bcol")
                nc.vector.tensor_sub(bcol[:], gb_sb[:, t : t + 1], tmb[:])
                Xt = p_X.tile([P, HW], F32R, tag="X", name=f"X_{b}_{t}")
                nc.vector.tensor_scalar(
                    out=Xt[:], in0=xt[t][:], scalar1=acol[:], scalar2=bcol[:],
                    op0=OP_MULT, op1=OP_ADD,
                )
                Xr.append(Xt)
            return xt, Xr

        gn_state = emit_gn(0)

        # ---- weights: DMA f32 staging -> ACT rounding copy -> f32r resident ----
        def load_wT(dram):
            t_r = p_const.tile([P, NT, C], F32R, tag=f"w_{dram.name}")
            r = dram.rearrange("(t p) o -> t p o", p=P)
            for ci in range(NT):
                st = p_stage.tile([P, C], F32, tag="wstage")
                nc.sync.dma_start(out=st[:], in_=r[ci])
                nc.scalar.copy(out=t_r[:, ci, :], in_=st[:])
            return t_r

        wq_r = load_wT(wqT_d)
        wk_r = load_wT(wkT_d)
        wv_r = load_wT(wvT_d)
        wo_r = load_wT(woT_d)

        ones_f = p_const.tile([P, P], F32, tag="ones_f")
        nc.sync.dma_start(out=ones_f[:], in_=ones_d)
        ones_r = p_const.tile([P, P], F32R, tag="ones_r")
        nc.scalar.copy(out=ones_r[:], in_=ones_f[:])

        # ---- per-image heavy phases ----
        for b in range(BL):
            xt, Xr = gn_state

            # Q^T / K^T projections (channel-major [o, n]); bias via ACT evac
            def proj_cm(w_r, bias_sb, tag, out_pool, bname):
                outs = []
                for ot in range(NT):
                    dst = out_pool.tile([P, HW], F32R, tag=tag, name=f"{bname}_{b}_{ot}")
                    for nch in range(NCH):
                        ps = ps_tile(f"ps_{bname}_{b}_{ot}_{nch}")
                        for ci in range(NT):
                            nc.tensor.matmul(
                                ps[:],
                                w_r[:, ci, ot * P : (ot + 1) * P],
                                Xr[ci][:, nch * FCH : (nch + 1) * FCH],
                                start=(ci == 0),
                                stop=(ci == NT - 1),
                            )
                        nc.scalar.activation(
                            out=dst[:, nch * FCH : (nch + 1) * FCH], in_=ps[:],
                            func=ACT_IDENT, bias=bias_sb[:, ot : ot + 1],
                        )
                    outs.append(dst)
                return outs

            QT = proj_cm(wq_r, bq_sb, "qt", p_qt, "q")
            KT = proj_cm(wk_r, bk_sb, "kt", p_kt, "k")

            # V projection (token-major [m, o]); bias bv folded into boP host-side
            Vr = []
            for mt in range(MT):
                ps = ps_tile(f"ps_v_{b}_{mt}")
                for ci in range(NT):
                    nc.tensor.matmul(
                        ps[:],
                        Xr[ci][:, mt * P : (mt + 1) * P],
                        wv_r[:, ci, :],
                        start=(ci == 0),
                        stop=(ci == NT - 1),
                    )
                vt = p_v.tile([P, C], F32R, tag="v", name=f"v_{b}_{mt}")
                nc.vector.tensor_copy(out=vt[:], in_=ps[:])
                Vr.append(vt)

            # scores S^T[m, n] -> exp -> column-sum denominators
            psC = []
            for _nch in range(NCH):
                psc_t = ps_tile(f"psc_{b}_{_nch}")
                psC.append(psc_t)
            expT = []
            for mt in range(MT):
                et = p_exp.tile([P, HW], F32R, tag="exp", name=f"e_{b}_{mt}")
                for nch in range(NCH):
                    psS = ps_tile(f"ps_s_{b}_{mt}_{nch}")
                    for ci in range(NT):
                        nc.tensor.matmul(
                            psS[:],
                            KT[ci][:, mt * P : (mt + 1) * P],
                            QT[ci][:, nch * FCH : (nch + 1) * FCH],
                            start=(ci == 0),
                            stop=(ci == NT - 1),
                        )
                    nc.scalar.activation(
                        out=et[:, nch * FCH : (nch + 1) * FCH], in_=psS[:],
                        func=ACT_EXP, scale=SCALE,
                    )
                    nc.tensor.matmul(
                        psC[nch][:],
                        ones_r[:],
                        et[:, nch * FCH : (nch + 1) * FCH],
                        start=(mt == 0),
                        stop=(mt == MT - 1),
                    )
                expT.append(et)

            recip = p_recip.tile([P, HW], F32, tag="recip", name=f"recip_{b}")
            for nch in range(NCH):
                nc.vector.reciprocal(
                    out=recip[:, nch * FCH : (nch + 1) * FCH], in_=psC[nch][:]
                )

            # GN for next image goes here so it schedules into this image's shadow
            if b + 1 < BL:
                gn_state = emit_gn(b + 1)

            # A^T[c, n] accumulated over m, normalized by 1/denom
            Ar = []
            for c2 in range(NT):
                at = p_a.tile([P, HW], F32R, tag="a", name=f"a_{b}_{c2}")
                for nch in range(NCH):
                    psA = ps_tile(f"ps_a_{b}_{c2}_{nch}")
                    for mt in range(MT):
                        nc.tensor.matmul(
                            psA[:],
                            Vr[mt][:, c2 * P : (c2 + 1) * P],
                            expT[mt][:, nch * FCH : (nch + 1) * FCH],
                            start=(mt == 0),
                            stop=(mt == MT - 1),
                        )
                    nc.vector.tensor_mul(
                        at[:, nch * FCH : (nch + 1) * FCH], psA[:],
                        recip[:, nch * FCH : (nch + 1) * FCH],
                    )
                Ar.append(at)

            # output projection + bias + residual
            for co in range(NT):
                for nch in range(NCH):
                    ps = ps_tile(f"ps_o_{b}_{co}_{nch}")
                    for oi in range(NT):
                        nc.tensor.matmul(
                            ps[:],
                            wo_r[:, oi, co * P : (co + 1) * P],
                            Ar[oi][:, nch * FCH : (nch + 1) * FCH],
                            start=(oi == 0),
                            stop=(oi == NT - 1),
                        )
                    ot = p_out.tile([P, FCH], F32, tag="out", name=f"o_{b}_{co}_{nch}")
                    nc.vector.scalar_tensor_tensor(
                        out=ot[:], in0=ps[:], scalar=boP_sb[:, co : co + 1],
                        in1=xt[co][:, nch * FCH : (nch + 1) * FCH],
                        op0=OP_ADD, op1=OP_ADD,
                    )
                    nc.sync.dma_start(
                        out=y_r[b, co][:, nch * FCH : (nch + 1) * FCH], in_=ot[:]
                    )

    nc.compile()
    return nc


def _host_inputs(x, gn_scale, gn_bias, wq, bq, wk, bk, wv, bv, wo, bo):
    f = lambda a: np.ascontiguousarray(np.asarray(a, dtype=np.float32))
    x = f(x).reshape(B, C, HW)
    boP = f(bo) + f(wo) @ f(bv)

    gm = np.zeros((P, NT, NGRP), np.float32)
    gmt = np.zeros((NGRP, NT, P), np.float32)
    for t in range(NT):
        for p in range(P):
            g = (t * P + p) // GS
            gm[p, t, g] = 1.0
            gmt[g, t, p] = 1.0
    ones = np.ones((P, P), np.float32)

    shared = {
        "wqT": np.ascontiguousarray(f(wq).T),
        "wkT": np.ascontiguousarray(f(wk).T),
        "wvT": np.ascontiguousarray(f(wv).T),
        "woT": np.ascontiguousarray(f(wo).T),
        "bq": f(bq), "bk": f(bk), "boP": boP,
        "gw": f(gn_scale), "gb": f(gn_bias),
        "gm": gm, "gmt": gmt, "ones": ones,
    }
    in_maps = []
    for i in range(N_CORES):
        m = dict(shared)
        m["x"] = np.ascontiguousarray(x[i * BL : (i + 1) * BL])
        in_maps.append(m)
    return in_maps


def kernel(x, gn_scale, gn_bias, wq, bq, wk, bk, wv, bv, wo, bo):
    global _CACHED_NC, LAST_EXEC_NS
    assert x.shape == (B, C, H, W)
    if _CACHED_NC is None:
        _CACHED_NC = _build_nc()
    in_maps = _host_inputs(x, gn_scale, gn_bias, wq, bq, wk, bk, wv, bv, wo, bo)
    trace = os.environ.get("ATT_TRACE", "0") == "1"
    kwargs = {}
    tdir = os.environ.get("ATT_TRACE_DIR")
    if tdir:
        kwargs["tmpdir"] = tdir
    res = run_bass_kernel_spmd(
        _CACHED_NC, in_maps, core_ids=list(range(N_CORES)), trace=trace, **kwargs
    )
    LAST_EXEC_NS = res.exec_time_ns
    y = np.concatenate([res.results[i]["y"] for i in range(N_CORES)], axis=0)
    return y.reshape(B, C, H, W).astype(np.float32)
